# revision 2
# baseline (speedup 1.0000x reference)
"""Trainium2 Bass kernel for nn_CrossAttentionLayer (sparse_attention).

Computation (per reference):
  q = (Wq @ x_slice + bq) / sqrt(C)            [C, H, W]
  k = Wk @ x_volume + bk                       [C, D, H, W]
  v = Wv @ x_volume + bv                       [C, D, H, W]
  att = softmax_over_D(q * k);  out = att * v  [C, D, H, W]

Sharding: H (=64) split across 8 cores, 8 rows each; every op is core-local.

Math notes:
  * bk drops out: q[c,hw]*bk[c] shifts logits uniformly along D, and softmax
    along D is shift-invariant, so k = Wk @ x_volume suffices.
  * no max-subtraction: |logits| <= ~0.6 by construction (weights ~0.05), so
    exp cannot overflow and the result matches the reference exactly.
  * softmax denominator S = sum_d E is accumulated on the TensorEngine via
    matmuls against a [128, 64] selector (partition p adds into column p%64),
    so the VectorEngine does only 3 streaming multiplies per element.

Per-core flow ("quarter" = 2 adjacent h-rows = 128 spatial cols x all 256 d):
  pass A (streaming x_volume once): per unit of 8 depths,
      kp = Wk @ xv           (2 matmuls into [128,512] PSUM, partitions =
                              channel x depth-group)
      kp *= q2               (DVE, q broadcast along depth via 0-stride AP)
      E_slice = exp(kp)      (ACT -> SBUF, quarter-resident [128, 16384])
      V_slice = vp + bv      (ACT Identity w/ per-partition bias -> SBUF)
      S += selector-matmuls over E_slice  (PE, PSUM accumulate)
  R = 1/S (transpose via PE, reciprocal on DVE), broadcast to [128, 128]
  pass B (pure SBUF): out = (E * V) * R, streamed out per unit.
"""

import sys

for _p in ("/opt/trn_rl_repo",):
    if _p not in sys.path:
        sys.path.append(_p)

import contextlib

import numpy as np

import concourse.bass as bass
import concourse.tile as tile
from concourse import mybir

F32 = mybir.dt.float32
FT = mybir.ActivationFunctionType

NCORES = 8
C = 64          # channels
D = 256         # depth
H = 64          # full height
W = 64          # width
HSH = H // NCORES               # h-rows per core = 8
COLS = D * HSH * W              # per-core flattened cols (d, h, w) = 2097152
NQ = HSH // 2                   # quarters per core = 4
QHW = 2 * W                     # spatial positions per quarter = 128
UCOLS = 1024                    # cols per unit (= 8 depths x 128 hw)
NU = (D * QHW) // UCOLS         # units per quarter = 32
BLK = 4                         # units per pass-B block


def split_excess_waits(nc, max_waits=1):
    """The walrus build in this container accepts at most `max_waits`
    sync-wait riders per instruction; move extras onto same-engine NoOps
    placed immediately before (same-engine order => semantics preserved)."""
    k = 0
    for bb in nc.main_func.blocks:
        insts = bb.instructions
        out = []
        changed = False
        for ins in insts:
            si = ins.sync_info
            if si is not None:
                waits = list(si.on_wait)
                if len(waits) > max_waits:
                    keep = waits[len(waits) - max_waits:]
                    for w in waits[: len(waits) - max_waits]:
                        k += 1
                        nop = mybir.InstNoOp(name=f"wsplit-{k}", engine=ins.engine)
                        nop.sync_info = mybir.SyncInfo(on_wait=[w], on_update=[])
                        nc.register_instruction(nop, overwrite=True)
                        out.append(nop)
                    si.on_wait = keep
                    changed = True
            out.append(ins)
        if changed:
            bb.instructions = out
    return k


def build_nc():
    nc = bass.Bass()
    xv = nc.dram_tensor("xv", [C, COLS], F32, kind="ExternalInput")
    xs = nc.dram_tensor("xs", [C + 1, HSH * W], F32, kind="ExternalInput")
    wq = nc.dram_tensor("wq", [C + 1, C], F32, kind="ExternalInput")
    wk = nc.dram_tensor("wk", [C, C], F32, kind="ExternalInput")
    wv = nc.dram_tensor("wv", [C, C], F32, kind="ExternalInput")
    bv2 = nc.dram_tensor("bv2", [128, 1], F32, kind="ExternalInput")
    sel = nc.dram_tensor("sel", [128, C], F32, kind="ExternalInput")
    iden = nc.dram_tensor("iden", [128, 128], F32, kind="ExternalInput")
    out = nc.dram_tensor("out", [C, COLS], F32, kind="ExternalOutput")

    xv3 = xv[:, :].rearrange("p (d hw) -> p d hw", hw=HSH * W)
    out3 = out[:, :].rearrange("p (d hw) -> p d hw", hw=HSH * W)

    with tile.TileContext(nc) as tc, contextlib.ExitStack() as ctx:
        const = ctx.enter_context(tc.tile_pool(name="const", bufs=1))
        xvp = ctx.enter_context(tc.tile_pool(name="xvp", bufs=6))
        ep = ctx.enter_context(tc.tile_pool(name="ep", bufs=1))
        vtp = ctx.enter_context(tc.tile_pool(name="vtp", bufs=1))
        qrp = ctx.enter_context(tc.tile_pool(name="qrp", bufs=2))
        outp = ctx.enter_context(tc.tile_pool(name="outp", bufs=3))
        misc = ctx.enter_context(tc.tile_pool(name="misc", bufs=2))
        kpp = ctx.enter_context(tc.tile_pool(name="kpp", bufs=2, space="PSUM"))
        vpp = ctx.enter_context(tc.tile_pool(name="vpp", bufs=2, space="PSUM"))
        stp = ctx.enter_context(tc.tile_pool(name="stp", bufs=1, space="PSUM"))
        tpp = ctx.enter_context(tc.tile_pool(name="tpp", bufs=1, space="PSUM"))

        wq_t = const.tile([C + 1, C], F32)
        nc.gpsimd.dma_start(wq_t[:], wq[:])
        wk_t = const.tile([C, C], F32)
        nc.gpsimd.dma_start(wk_t[:], wk[:])
        wv_t = const.tile([C, C], F32)
        nc.gpsimd.dma_start(wv_t[:], wv[:])
        bv2_t = const.tile([128, 1], F32)
        nc.gpsimd.dma_start(bv2_t[:], bv2[:])
        sel_t = const.tile([128, C], F32)
        nc.gpsimd.dma_start(sel_t[:], sel[:])
        iden_t = const.tile([128, 128], F32)
        nc.gpsimd.dma_start(iden_t[:], iden[:])
        xs_t = const.tile([C + 1, HSH * W], F32)
        nc.gpsimd.dma_start(xs_t[:], xs[:])

        # q_s = (Wq @ xs + bq) / sqrt(C)   (bias via ones-row of xs / row 64 of wq)
        q_ps = tpp.tile([C, HSH * W], F32, tag="qps")
        nc.tensor.matmul(q_ps[:], wq_t[:], xs_t[:], start=True, stop=True)
        q_s = const.tile([C, HSH * W], F32)
        nc.scalar.activation(q_s[:], q_ps[:], FT.Copy, bias=0.0,
                             scale=1.0 / float(np.sqrt(C)))

        for q in range(NQ):
            # q2: [128, 128] — this quarter's q_s, duplicated on both halves.
            q2 = qrp.tile([128, QHW], F32, tag="q2")
            qsl = q_s[:, q * QHW:(q + 1) * QHW]
            nc.gpsimd.dma_start(q2[0:C, :], qsl)
            nc.gpsimd.dma_start(q2[C:128, :], qsl)

            e_t = ep.tile([128, NU * 512], F32, tag="E")
            v_t = vtp.tile([128, NU * 512], F32, tag="V")
            st = stp.tile([128, C], F32, tag="st")

            # ---- pass A
            for t in range(NU):
                xt = xvp.tile([C, UCOLS], F32, tag="xt")
                nc.gpsimd.dma_start(
                    xt[:].rearrange("p (d hw) -> p d hw", hw=QHW),
                    xv3[:, 8 * t:8 * (t + 1), q * QHW:(q + 1) * QHW])
                kp_t = kpp.tile([128, 512], F32, tag="kp")
                nc.tensor.matmul(kp_t[0:C, :], wk_t[:], xt[:, 0:512],
                                 start=True, stop=True)
                nc.tensor.matmul(kp_t[C:128, :], wk_t[:], xt[:, 512:1024],
                                 start=True, stop=True)
                vp_t = vpp.tile([128, 512], F32, tag="vp")
                nc.tensor.matmul(vp_t[0:C, :], wv_t[:], xt[:, 0:512],
                                 start=True, stop=True)
                nc.tensor.matmul(vp_t[C:128, :], wv_t[:], xt[:, 512:1024],
                                 start=True, stop=True)
                kp3 = kp_t[:].rearrange("p (d hw) -> p d hw", hw=QHW)
                nc.vector.tensor_mul(
                    kp3, kp3, q2[:].unsqueeze(1).broadcast_to([128, 4, QHW]))
                nc.scalar.activation(e_t[:, t * 512:(t + 1) * 512], kp_t[:], FT.Exp)
                # V~ = vp + bv (per-partition bias rides the PSUM->SBUF copy)
                nc.scalar.activation(v_t[:, t * 512:(t + 1) * 512], vp_t[:],
                                     FT.Identity, bias=bv2_t[:, 0:1])
                for d2 in range(4):
                    nc.tensor.matmul(
                        st[:], e_t[:, t * 512 + d2 * QHW:t * 512 + (d2 + 1) * QHW],
                        sel_t[:],
                        start=(t == 0 and d2 == 0), stop=(t == NU - 1 and d2 == 3),
                        skip_group_check=True)

            # ---- R = 1/S broadcast to [128, 128]
            st_sb = misc.tile([128, C], F32, tag="stsb")
            nc.scalar.copy(st_sb[:], st[:])
            stT = tpp.tile([C, 128], F32, tag="stT")
            nc.tensor.transpose(stT[:], st_sb[:], iden_t[:])
            r_sb = misc.tile([C, QHW], F32, tag="rsb")
            nc.vector.reciprocal(r_sb[:], stT[:])
            r2 = qrp.tile([128, QHW], F32, tag="r2")
            nc.gpsimd.dma_start(r2[0:C, :], r_sb[:])
            nc.gpsimd.dma_start(r2[C:128, :], r_sb[:])

            # ---- pass B: out = (E * V) * R  (pure SBUF streaming)
            for b in range(NU // BLK):
                lo, hi = b * BLK * 512, (b + 1) * BLK * 512
                nrep = BLK * 4
                vb = v_t[:, lo:hi].rearrange("p (d hw) -> p d hw", hw=QHW)
                eb = e_t[:, lo:hi].rearrange("p (d hw) -> p d hw", hw=QHW)
                nc.vector.tensor_mul(vb, vb, eb)
                o_sb = outp.tile([128, BLK * 512], F32, tag="o")
                o3 = o_sb[:].rearrange("p (d hw) -> p d hw", hw=QHW)
                nc.vector.tensor_mul(
                    o3, vb, r2[:].unsqueeze(1).broadcast_to([128, nrep, QHW]))
                for u in range(BLK):
                    t = b * BLK + u
                    ou = o_sb[:, u * 512:(u + 1) * 512]
                    nc.gpsimd.dma_start(
                        out3[:, 8 * t:8 * t + 4, q * QHW:(q + 1) * QHW],
                        ou[0:C, :].rearrange("p (d hw) -> p d hw", hw=QHW))
                    nc.gpsimd.dma_start(
                        out3[:, 8 * t + 4:8 * t + 8, q * QHW:(q + 1) * QHW],
                        ou[C:128, :].rearrange("p (d hw) -> p d hw", hw=QHW))

    split_excess_waits(nc)
    return nc


def make_in_maps(x_slice, x_volume, Wq, bq, Wk, bk, Wv, bv):
    x_slice = np.asarray(x_slice, dtype=np.float32)
    x_volume = np.asarray(x_volume, dtype=np.float32)
    wq65 = np.ascontiguousarray(
        np.concatenate([np.asarray(Wq, np.float32).T,
                        np.asarray(bq, np.float32)[None, :]], axis=0))
    wkT = np.ascontiguousarray(np.asarray(Wk, np.float32).T)
    wvT = np.ascontiguousarray(np.asarray(Wv, np.float32).T)
    bv_np = np.asarray(bv, np.float32)
    bv2 = np.ascontiguousarray(np.concatenate([bv_np, bv_np])[:, None])
    sel = np.zeros((128, C), np.float32)
    sel[np.arange(128), np.arange(128) % C] = 1.0
    iden = np.eye(128, dtype=np.float32)

    in_maps = []
    for i in range(NCORES):
        h0 = i * HSH
        xv_i = np.ascontiguousarray(
            x_volume[0, :, :, h0:h0 + HSH, :].reshape(C, COLS))
        xs_i = np.empty((C + 1, HSH * W), np.float32)
        xs_i[:C] = x_slice[0, :, h0:h0 + HSH, :].reshape(C, HSH * W)
        xs_i[C] = 1.0
        in_maps.append({"xv": xv_i, "xs": xs_i, "wq": wq65, "wk": wkT,
                        "wv": wvT, "bv2": bv2, "sel": sel, "iden": iden})
    return in_maps


def assemble_output(results):
    full = np.empty((1, C, D, H, W), np.float32)
    for i in range(NCORES):
        full[0, :, :, i * HSH:(i + 1) * HSH, :] = \
            results[i]["out"].reshape(C, D, HSH, W)
    return full


_NC = None


def _get_nc():
    global _NC
    if _NC is None:
        _NC = build_nc()
    return _NC


def kernel(x_slice, x_volume, Wq, bq, Wk, bk, Wv, bv):
    from concourse.bass_utils import run_bass_kernel_spmd

    nc = _get_nc()
    in_maps = make_in_maps(x_slice, x_volume, Wq, bq, Wk, bk, Wv, bv)
    r = run_bass_kernel_spmd(nc, in_maps, list(range(NCORES)), trace=False)
    return assemble_output(r.results)


# revision 3
# speedup vs baseline: 1.0008x; 1.0008x over previous
"""Trainium2 Bass kernel for nn_CrossAttentionLayer (sparse_attention).

Computation (per reference):
  q = (Wq @ x_slice + bq) / sqrt(C)            [C, H, W]
  k = Wk @ x_volume + bk                       [C, D, H, W]
  v = Wv @ x_volume + bv                       [C, D, H, W]
  att = softmax_over_D(q * k);  out = att * v  [C, D, H, W]

Sharding: H (=64) split across 8 cores, 8 rows each; every op is core-local.

Math notes:
  * bk drops out: q[c,hw]*bk[c] shifts logits uniformly along D, and softmax
    along D is shift-invariant, so k = Wk @ x_volume suffices.
  * no max-subtraction: |logits| <= ~0.6 by construction (weights ~0.05), so
    exp cannot overflow and the result matches the reference exactly.
  * softmax denominator S = sum_d E is accumulated on the TensorEngine via
    matmuls against a [128, 64] selector (partition p adds into column p%64),
    so the VectorEngine does only 3 streaming multiplies per element.

Host-side data layout: the per-core x_volume shard is pre-permuted to
[C, (quarter, depth, 2, W)] ("quarter" = 2 adjacent h-rows = 128 spatial
positions), so every device DMA is contiguous per partition.  The output
uses the same layout and is permuted back on the host.

Per-core flow per quarter:
  pass A (streaming x_volume once): per unit of 8 depths,
      kp = Wk @ xv           (2 matmuls into [128,512] PSUM, partitions =
                              channel x depth-group)
      kp *= q2               (DVE, q broadcast along depth via 0-stride AP)
      E_slice = exp(kp)      (ACT -> SBUF, quarter-resident [128, 16384])
      V_slice = vp + bv      (ACT Identity w/ per-partition bias -> SBUF)
      S += selector-matmuls over E_slice  (PE, PSUM accumulate)
  R = 1/S (transpose via PE, reciprocal on DVE), broadcast to [128, 128]
  pass B (pure SBUF): out = (E * V) * R, streamed out per 4-unit block.
"""

import sys

for _p in ("/opt/trn_rl_repo",):
    if _p not in sys.path:
        sys.path.append(_p)

import contextlib

import numpy as np

import concourse.bass as bass
import concourse.tile as tile
from concourse import mybir

F32 = mybir.dt.float32
FT = mybir.ActivationFunctionType

NCORES = 8
C = 64          # channels
D = 256         # depth
H = 64          # full height
W = 64          # width
HSH = H // NCORES               # h-rows per core = 8
COLS = D * HSH * W              # per-core flattened cols = 131072
NQ = HSH // 2                   # quarters per core = 4
QHW = 2 * W                     # spatial positions per quarter = 128
QCOLS = D * QHW                 # cols per quarter = 32768
UCOLS = 1024                    # cols per unit (= 8 depths x 128 hw)
NU = QCOLS // UCOLS             # units per quarter = 32
BLK = 4                         # units per pass-B block


def split_excess_waits(nc, max_waits=1):
    """The walrus build in this container accepts at most `max_waits`
    sync-wait riders per instruction; move extras onto same-engine NoOps
    placed immediately before (same-engine order => semantics preserved)."""
    k = 0
    for bb in nc.main_func.blocks:
        insts = bb.instructions
        out = []
        changed = False
        for ins in insts:
            si = ins.sync_info
            if si is not None:
                waits = list(si.on_wait)
                if len(waits) > max_waits:
                    keep = waits[len(waits) - max_waits:]
                    for w in waits[: len(waits) - max_waits]:
                        k += 1
                        nop = mybir.InstNoOp(name=f"wsplit-{k}", engine=ins.engine)
                        nop.sync_info = mybir.SyncInfo(on_wait=[w], on_update=[])
                        nc.register_instruction(nop, overwrite=True)
                        out.append(nop)
                    si.on_wait = keep
                    changed = True
            out.append(ins)
        if changed:
            bb.instructions = out
    return k


def build_nc(nq=NQ):
    nc = bass.Bass()
    xv = nc.dram_tensor("xv", [C, COLS], F32, kind="ExternalInput")
    xs = nc.dram_tensor("xs", [C + 1, HSH * W], F32, kind="ExternalInput")
    wq = nc.dram_tensor("wq", [C + 1, C], F32, kind="ExternalInput")
    wk = nc.dram_tensor("wk", [C, C], F32, kind="ExternalInput")
    wv = nc.dram_tensor("wv", [C, C], F32, kind="ExternalInput")
    bv2 = nc.dram_tensor("bv2", [128, 1], F32, kind="ExternalInput")
    sel = nc.dram_tensor("sel", [128, C], F32, kind="ExternalInput")
    iden = nc.dram_tensor("iden", [128, 128], F32, kind="ExternalInput")
    out = nc.dram_tensor("out", [C, COLS], F32, kind="ExternalOutput")

    with tile.TileContext(nc) as tc, contextlib.ExitStack() as ctx:
        const = ctx.enter_context(tc.tile_pool(name="const", bufs=1))
        xvp = ctx.enter_context(tc.tile_pool(name="xvp", bufs=6))
        ep = ctx.enter_context(tc.tile_pool(name="ep", bufs=1))
        vtp = ctx.enter_context(tc.tile_pool(name="vtp", bufs=1))
        qrp = ctx.enter_context(tc.tile_pool(name="qrp", bufs=2))
        outp = ctx.enter_context(tc.tile_pool(name="outp", bufs=3))
        misc = ctx.enter_context(tc.tile_pool(name="misc", bufs=2))
        kpp = ctx.enter_context(tc.tile_pool(name="kpp", bufs=2, space="PSUM"))
        vpp = ctx.enter_context(tc.tile_pool(name="vpp", bufs=2, space="PSUM"))
        stp = ctx.enter_context(tc.tile_pool(name="stp", bufs=1, space="PSUM"))
        tpp = ctx.enter_context(tc.tile_pool(name="tpp", bufs=1, space="PSUM"))

        wq_t = const.tile([C + 1, C], F32)
        nc.gpsimd.dma_start(wq_t[:], wq[:])
        wk_t = const.tile([C, C], F32)
        nc.gpsimd.dma_start(wk_t[:], wk[:])
        wv_t = const.tile([C, C], F32)
        nc.gpsimd.dma_start(wv_t[:], wv[:])
        bv2_t = const.tile([128, 1], F32)
        nc.gpsimd.dma_start(bv2_t[:], bv2[:])
        sel_t = const.tile([128, C], F32)
        nc.gpsimd.dma_start(sel_t[:], sel[:])
        iden_t = const.tile([128, 128], F32)
        nc.gpsimd.dma_start(iden_t[:], iden[:])
        xs_t = const.tile([C + 1, HSH * W], F32)
        nc.gpsimd.dma_start(xs_t[:], xs[:])

        # q_s = (Wq @ xs + bq) / sqrt(C)   (bias via ones-row of xs / row 64 of wq)
        q_ps = tpp.tile([C, HSH * W], F32, tag="qps")
        nc.tensor.matmul(q_ps[:], wq_t[:], xs_t[:], start=True, stop=True)
        q_s = const.tile([C, HSH * W], F32)
        nc.scalar.activation(q_s[:], q_ps[:], FT.Copy, bias=0.0,
                             scale=1.0 / float(np.sqrt(C)))

        for q in range(nq):
            qb = q * QCOLS
            # q2: [128, 128] — this quarter's q_s, duplicated on both halves.
            q2 = qrp.tile([128, QHW], F32, tag="q2")
            qsl = q_s[:, q * QHW:(q + 1) * QHW]
            nc.gpsimd.dma_start(q2[0:C, :], qsl)
            nc.gpsimd.dma_start(q2[C:128, :], qsl)

            e_t = ep.tile([128, NU * 512], F32, tag="E")
            v_t = vtp.tile([128, NU * 512], F32, tag="V")
            st = stp.tile([128, C], F32, tag="st")

            # ---- pass A
            for t in range(NU):
                xt = xvp.tile([C, UCOLS], F32, tag="xt")
                nc.gpsimd.dma_start(xt[:], xv[:, qb + t * UCOLS:qb + (t + 1) * UCOLS])
                kp_t = kpp.tile([128, 512], F32, tag="kp")
                nc.tensor.matmul(kp_t[0:C, :], wk_t[:], xt[:, 0:512],
                                 start=True, stop=True)
                nc.tensor.matmul(kp_t[C:128, :], wk_t[:], xt[:, 512:1024],
                                 start=True, stop=True)
                vp_t = vpp.tile([128, 512], F32, tag="vp")
                nc.tensor.matmul(vp_t[0:C, :], wv_t[:], xt[:, 0:512],
                                 start=True, stop=True)
                nc.tensor.matmul(vp_t[C:128, :], wv_t[:], xt[:, 512:1024],
                                 start=True, stop=True)
                kp3 = kp_t[:].rearrange("p (d hw) -> p d hw", hw=QHW)
                nc.vector.tensor_mul(
                    kp3, kp3, q2[:].unsqueeze(1).broadcast_to([128, 4, QHW]))
                nc.scalar.activation(e_t[:, t * 512:(t + 1) * 512], kp_t[:], FT.Exp)
                # V~ = vp + bv (per-partition bias rides the PSUM->SBUF copy)
                nc.scalar.activation(v_t[:, t * 512:(t + 1) * 512], vp_t[:],
                                     FT.Identity, bias=bv2_t[:, 0:1])
                for d2 in range(4):
                    nc.tensor.matmul(
                        st[:], e_t[:, t * 512 + d2 * QHW:t * 512 + (d2 + 1) * QHW],
                        sel_t[:],
                        start=(t == 0 and d2 == 0), stop=(t == NU - 1 and d2 == 3),
                        skip_group_check=True)

            # ---- R = 1/S broadcast to [128, 128]
            st_sb = misc.tile([128, C], F32, tag="stsb")
            nc.scalar.copy(st_sb[:], st[:])
            stT = tpp.tile([C, 128], F32, tag="stT")
            nc.tensor.transpose(stT[:], st_sb[:], iden_t[:])
            r_sb = misc.tile([C, QHW], F32, tag="rsb")
            nc.vector.reciprocal(r_sb[:], stT[:])
            r2 = qrp.tile([128, QHW], F32, tag="r2")
            nc.gpsimd.dma_start(r2[0:C, :], r_sb[:])
            nc.gpsimd.dma_start(r2[C:128, :], r_sb[:])

            # ---- pass B: out = (E * V) * R  (pure SBUF streaming)
            for b in range(NU // BLK):
                lo, hi = b * BLK * 512, (b + 1) * BLK * 512
                nrep = BLK * 4
                vb = v_t[:, lo:hi].rearrange("p (d hw) -> p d hw", hw=QHW)
                eb = e_t[:, lo:hi].rearrange("p (d hw) -> p d hw", hw=QHW)
                nc.vector.tensor_mul(vb, vb, eb)
                o_sb = outp.tile([128, BLK * 512], F32, tag="o")
                o3 = o_sb[:].rearrange("p (d hw) -> p d hw", hw=QHW)
                nc.vector.tensor_mul(
                    o3, vb, r2[:].unsqueeze(1).broadcast_to([128, nrep, QHW]))
                # store block: one DMA per depth-group half, strided over units
                dstb = out[:, qb + b * BLK * UCOLS:qb + (b + 1) * BLK * UCOLS]
                dst4 = dstb.rearrange("p (u g s) -> p u g s", g=2, s=512)
                src4 = o_sb[:].rearrange("p (u s) -> p u s", s=512)
                nc.gpsimd.dma_start(dst4[:, :, 0, :], src4[0:C, :, :])
                nc.gpsimd.dma_start(dst4[:, :, 1, :], src4[C:128, :, :])

    split_excess_waits(nc)
    return nc


def make_in_maps(x_slice, x_volume, Wq, bq, Wk, bk, Wv, bv):
    x_slice = np.asarray(x_slice, dtype=np.float32)
    x_volume = np.asarray(x_volume, dtype=np.float32)
    wq65 = np.ascontiguousarray(
        np.concatenate([np.asarray(Wq, np.float32).T,
                        np.asarray(bq, np.float32)[None, :]], axis=0))
    wkT = np.ascontiguousarray(np.asarray(Wk, np.float32).T)
    wvT = np.ascontiguousarray(np.asarray(Wv, np.float32).T)
    bv_np = np.asarray(bv, np.float32)
    bv2 = np.ascontiguousarray(np.concatenate([bv_np, bv_np])[:, None])
    sel = np.zeros((128, C), np.float32)
    sel[np.arange(128), np.arange(128) % C] = 1.0
    iden = np.eye(128, dtype=np.float32)

    in_maps = []
    for i in range(NCORES):
        h0 = i * HSH
        # [C, D, HSH, W] -> [C, (q, d, r, w)] with h = 2q + r
        shard = x_volume[0, :, :, h0:h0 + HSH, :].reshape(C, D, NQ, 2, W)
        xv_i = np.ascontiguousarray(
            shard.transpose(0, 2, 1, 3, 4).reshape(C, COLS))
        xs_i = np.empty((C + 1, HSH * W), np.float32)
        xs_i[:C] = x_slice[0, :, h0:h0 + HSH, :].reshape(C, HSH * W)
        xs_i[C] = 1.0
        in_maps.append({"xv": xv_i, "xs": xs_i, "wq": wq65, "wk": wkT,
                        "wv": wvT, "bv2": bv2, "sel": sel, "iden": iden})
    return in_maps


def assemble_output(results):
    full = np.empty((1, C, D, H, W), np.float32)
    for i in range(NCORES):
        o = results[i]["out"].reshape(C, NQ, D, 2, W).transpose(0, 2, 1, 3, 4)
        full[0, :, :, i * HSH:(i + 1) * HSH, :] = o.reshape(C, D, HSH, W)
    return full


_NC = None


def _get_nc():
    global _NC
    if _NC is None:
        _NC = build_nc()
    return _NC


def kernel(x_slice, x_volume, Wq, bq, Wk, bk, Wv, bv):
    from concourse.bass_utils import run_bass_kernel_spmd

    nc = _get_nc()
    in_maps = make_in_maps(x_slice, x_volume, Wq, bq, Wk, bk, Wv, bv)
    r = run_bass_kernel_spmd(nc, in_maps, list(range(NCORES)), trace=False)
    return assemble_output(r.results)
